# revision 9
# baseline (speedup 1.0000x reference)
"""Trainium2 Bass kernel for nn_AttCNN4Weight (sparse_attention).

Data-parallel over batch: each of the 8 NeuronCores handles 8 of the 64
batch elements end-to-end (dynamic per-sample conv kernel -> sliding-window
score -> masked softmax over kv_len -> weighted sum of v). No collectives.

Host-side work is layout only: batch sharding, transposes so every DMA
moves multi-KB contiguous rows, a column reorder of W to (tap, channel)
order, and a bf16 cast of v (the attend reduction tolerates bf16 easily;
halves v HBM traffic).

Key trick: the KW=3 sliding-window score is a single matmul contraction
over (tap, channel) = 1536, where each tap's k-operand is just a +/-1
shifted free-dim slice of the same padded k tile. That keeps every matmul
output at PSUM partition base 0 (a hardware requirement) with M=1.
"""

import sys

if "/opt/trn_rl_repo" not in sys.path:
    sys.path.insert(0, "/opt/trn_rl_repo")

import numpy as np
from contextlib import ExitStack

L, B, C, Q, V, KW = 2048, 64, 512, 512, 512, 3
NCORES = 8
BC = B // NCORES          # 8 batch elements per core
M12 = KW * (C // 128)     # 12 contraction chunks of (tap, channel)
NEGBIG = 3.0e38           # additive mask constant (finite, exp() underflows to 0)

_NC = None


def _build():
    import concourse.bacc as bacc
    import concourse.tile as tile
    from concourse import mybir
    from concourse.masks import make_identity

    f32 = mybir.dt.float32
    bf16 = mybir.dt.bfloat16
    i32 = mybir.dt.int32

    nc = bacc.Bacc(None)

    kT = nc.declare_dram_parameter("kT", [BC, C, L], f32, isOutput=False)
    vT = nc.declare_dram_parameter("vT", [BC, L, V], bf16, isOutput=False)
    mT = nc.declare_dram_parameter("mT", [BC, L], i32, isOutput=False)
    qT = nc.declare_dram_parameter("qT", [Q, BC], f32, isOutput=False)
    Wr = nc.declare_dram_parameter("Wr", [Q, KW * C], f32, isOutput=False)
    Br = nc.declare_dram_parameter("Br", [128, M12], f32, isOutput=False)
    a_out = nc.declare_dram_parameter("a_out", [BC, L], f32, isOutput=True)
    e_out = nc.declare_dram_parameter("e_out", [BC, L], f32, isOutput=True)
    t_out = nc.declare_dram_parameter("t_out", [BC, V], f32, isOutput=True)

    with ExitStack() as ctx:
        tc = ctx.enter_context(tile.TileContext(nc))
        singles = ctx.enter_context(tc.tile_pool(name="singles", bufs=1))
        sa = ctx.enter_context(tc.tile_pool(name="sa", bufs=2))
        kpool = ctx.enter_context(tc.tile_pool(name="kpool", bufs=8))
        vpool = ctx.enter_context(tc.tile_pool(name="vpool", bufs=9))
        pq = ctx.enter_context(tc.tile_pool(name="pq", bufs=1, space="PSUM"))
        pcv = ctx.enter_context(tc.tile_pool(name="pcv", bufs=4, space="PSUM"))
        ptr = ctx.enter_context(tc.tile_pool(name="ptr", bufs=1, space="PSUM"))
        pat = ctx.enter_context(tc.tile_pool(name="pat", bufs=2, space="PSUM"))

        # ---- replicated params / small tensors ----
        wr_sb = singles.tile([128, Q // 128, KW * C], f32, tag="wr")
        q_sb = singles.tile([128, Q // 128, BC], f32, tag="q")
        b_sb = singles.tile([128, M12], f32, tag="bias")
        kern = singles.tile([128, M12, BC], f32, tag="kern")
        ident = singles.tile([128, 128], f32, tag="ident")
        maskf = singles.tile([BC, L], f32, tag="maskf")
        A_sb = singles.tile([BC, L], f32, tag="a")
        Am = singles.tile([BC, L], f32, tag="am")   # becomes e_ij in place
        E_sb = Am
        ET = singles.tile([128, L // 128, BC], bf16, tag="et")
        nmx = singles.tile([BC, 1], f32, tag="nmx")
        ssum = singles.tile([BC, 1], f32, tag="ssum")
        sinv = singles.tile([BC, 1], f32, tag="sinv")
        # mask-int tile borrows a k-pool slot (same byte size, freed after cast)
        msk_i = kpool.tile([BC, L], i32, tag="k")

        nc.sync.dma_start(out=wr_sb, in_=Wr[:].rearrange("(qc p) n -> p qc n", p=128))
        nc.sync.dma_start(out=q_sb, in_=qT[:].rearrange("(qc p) b -> p qc b", p=128))
        nc.sync.dma_start(out=b_sb, in_=Br[:])
        nc.sync.dma_start(out=msk_i, in_=mT[:])
        make_identity(nc, ident)

        # mask -> f32; additive mask: (m-1)*NEGBIG in {0, -NEGBIG} (in place)
        nc.vector.tensor_copy(out=maskf, in_=msk_i)
        nc.vector.tensor_scalar(
            out=maskf, in0=maskf, scalar1=-1.0, scalar2=NEGBIG,
            op0=mybir.AluOpType.add, op1=mybir.AluOpType.mult,
        )

        # ---- kern[p, m, b] = (q @ W.T + b) in (tap, channel) order ----
        for m in range(M12):
            pqt = pq.tile([128, BC], f32, tag="pq")
            for qc in range(Q // 128):
                nc.tensor.matmul(
                    pqt,
                    wr_sb[:, qc, m * 128:(m + 1) * 128],
                    q_sb[:, qc, :],
                    start=(qc == 0), stop=(qc == Q // 128 - 1),
                )
            nc.vector.tensor_scalar_add(
                out=kern[:, m, :], in0=pqt, scalar1=b_sb[:, m:m + 1]
            )

        # ---- a_ij[l, b] = sum_{w,c} k[l+w-1, b, c] * kern[b, c, w] ----
        # k tiles are padded by one zero column on each side so tap shifts
        # are plain free-dim offsets. m-chunk (w, cc) pairs kern[:, m, b]
        # with k rows cc shifted by (w - 1).
        for b_ in range(BC):
            ksb = []
            for cc in range(C // 128):
                kt = kpool.tile([128, L + 2], f32, tag="k")
                nc.vector.memset(kt[:, 0:1], 0.0)
                nc.vector.memset(kt[:, L + 1:L + 2], 0.0)
                nc.sync.dma_start(
                    out=kt[:, 1:L + 1], in_=kT[b_, cc * 128:(cc + 1) * 128, :]
                )
                ksb.append(kt)
            arow = sa.tile([1, L], f32, tag="arow")
            for lc in range(L // 512):
                cv = pcv.tile([1, 512], f32, tag="cv")
                for m in range(M12):
                    w, cc = m // 4, m % 4
                    nc.tensor.matmul(
                        cv,
                        kern[:, m, b_:b_ + 1],
                        ksb[cc][:, lc * 512 + w:lc * 512 + w + 512],
                        start=(m == 0), stop=(m == M12 - 1),
                    )
                nc.scalar.copy(out=arow[:, lc * 512:(lc + 1) * 512], in_=cv)
            # partition-scatter: land this batch row at partition b_ (DMA only)
            nc.sync.dma_start(out=A_sb[b_:b_ + 1, :], in_=arow)

        # ---- masked softmax over l (b on partitions 0..7) ----
        nc.vector.tensor_add(out=Am, in0=A_sb, in1=maskf)
        nc.vector.tensor_reduce(
            out=nmx, in_=Am, op=mybir.AluOpType.max,
            axis=mybir.AxisListType.X, negate=True,
        )
        nc.scalar.activation(
            out=E_sb, in_=Am, func=mybir.ActivationFunctionType.Exp,
            bias=nmx[:, 0:1], scale=1.0, accum_out=ssum,
        )
        nc.vector.reciprocal(out=sinv, in_=ssum)
        nc.vector.tensor_scalar_mul(out=E_sb, in0=E_sb, scalar1=sinv[:, 0:1])

        nc.sync.dma_start(out=a_out[:], in_=A_sb)
        nc.sync.dma_start(out=e_out[:], in_=E_sb)

        # ---- ET[p, lt, b] = E[b, lt*128+p] (PE transpose, cast to bf16) ----
        for lt in range(L // 128):
            trp = ptr.tile([128, BC], f32, tag="tr")
            nc.tensor.transpose(trp, E_sb[:, lt * 128:(lt + 1) * 128], ident[0:BC, 0:BC])
            nc.vector.tensor_copy(out=ET[:, lt, :], in_=trp)

        # ---- attend[b, :] = sum_l e[l, b] * v[l, b, :] ----
        # lhsT = ET[:, lt, :] gives an [8, 512] output whose row m pairs
        # e(:, m) with v(:, b); only row b is real — DMA just that row out.
        for b_ in range(BC):
            vsb = []
            for jj in range(2):
                vt = vpool.tile([128, 8, V], bf16, tag="v")
                nc.sync.dma_start(
                    out=vt,
                    in_=vT[b_, jj * 1024:(jj + 1) * 1024, :].rearrange(
                        "(i p) v -> p i v", p=128
                    ),
                )
                vsb.append(vt)
            att_ps = pat.tile([BC, V], f32, tag="atp")
            for lt in range(L // 128):
                nc.tensor.matmul(
                    att_ps,
                    ET[:, lt, :],
                    vsb[lt // 8][:, lt % 8, :],
                    start=(lt == 0), stop=(lt == L // 128 - 1),
                )
            att_sb = sa.tile([BC, V], f32, tag="attsb")
            nc.vector.tensor_copy(out=att_sb, in_=att_ps)
            nc.sync.dma_start(out=t_out[b_:b_ + 1, :], in_=att_sb[b_:b_ + 1, :])

    nc.compile()
    return nc


def get_nc():
    global _NC
    if _NC is None:
        _NC = _build()
    return _NC


def make_in_maps(q, k, v, k_mask, W, b):
    import ml_dtypes

    q = np.ascontiguousarray(np.asarray(q, dtype=np.float32))
    k = np.asarray(k, dtype=np.float32)
    v = np.asarray(v, dtype=np.float32)
    k_mask = np.asarray(k_mask, dtype=np.int32)
    W = np.asarray(W, dtype=np.float32)
    b = np.asarray(b, dtype=np.float32)

    # W column reorder: Wr[q, w*C + c] = W[c*KW + w, q]
    Wr = np.ascontiguousarray(W.reshape(C, KW, Q).transpose(2, 1, 0).reshape(Q, KW * C))
    Br = np.ascontiguousarray(b.reshape(C, KW).T.reshape(KW * C).reshape(M12, 128).T)

    in_maps = []
    for i in range(NCORES):
        sl = slice(i * BC, (i + 1) * BC)
        in_maps.append({
            "kT": np.ascontiguousarray(k[:, sl, :].transpose(1, 2, 0)),
            "vT": np.ascontiguousarray(v[:, sl, :].transpose(1, 0, 2)).astype(
                ml_dtypes.bfloat16
            ),
            "mT": np.ascontiguousarray(k_mask[:, sl].T),
            "qT": np.ascontiguousarray(q[sl, :].T),
            "Wr": Wr,
            "Br": Br,
        })
    return in_maps


def assemble(results):
    a = np.concatenate([r["a_out"] for r in results], axis=0).T.copy()
    e = np.concatenate([r["e_out"] for r in results], axis=0).T.copy()
    att = np.concatenate([r["t_out"] for r in results], axis=0)
    return (
        np.ascontiguousarray(a, dtype=np.float32),
        np.ascontiguousarray(e, dtype=np.float32),
        np.ascontiguousarray(att, dtype=np.float32),
    )


def kernel(q, k, v, k_mask, W, b):
    from concourse.bass_utils import run_bass_kernel_spmd

    nc = get_nc()
    in_maps = make_in_maps(q, k, v, k_mask, W, b)
    res = run_bass_kernel_spmd(nc, in_maps, core_ids=list(range(NCORES)))
    return assemble(res.results)


# revision 14
# speedup vs baseline: 1.6221x; 1.6221x over previous
"""Trainium2 Bass kernel for nn_AttCNN4Weight (sparse_attention).

Data-parallel over batch: each of the 8 NeuronCores handles 8 of the 64
batch elements end-to-end (dynamic per-sample conv kernel -> sliding-window
score -> masked softmax over kv_len -> weighted sum of v). No collectives.

Host-side work is layout only: batch sharding, transposes so every DMA
moves multi-KB contiguous rows, a column reorder of W to (tap, channel)
order, and a bf16 cast of v (the attend reduction tolerates bf16 easily;
halves v HBM traffic).

Key trick: the KW=3 sliding-window score is a single matmul contraction
over (tap, channel) = 1536, where each tap's k-operand is just a +/-1
shifted free-dim slice of the same padded k tile. That keeps every matmul
output at PSUM partition base 0 (a hardware requirement) with M=1.
"""

import sys

if "/opt/trn_rl_repo" not in sys.path:
    sys.path.insert(0, "/opt/trn_rl_repo")

import numpy as np
from contextlib import ExitStack

L, B, C, Q, V, KW = 2048, 64, 512, 512, 512, 3
NCORES = 8
BC = B // NCORES          # 8 batch elements per core
M12 = KW * (C // 128)     # 12 contraction chunks of (tap, channel)
NEGBIG = 3.0e38           # additive mask constant (finite, exp() underflows to 0)

_NC = None


def _build():
    import concourse.bass as bass
    import concourse.bacc as bacc
    import concourse.tile as tile
    from concourse import mybir
    from concourse.masks import make_identity

    f32 = mybir.dt.float32
    bf16 = mybir.dt.bfloat16
    i32 = mybir.dt.int32

    nc = bacc.Bacc(None)

    kT = nc.declare_dram_parameter("kT", [BC, C, L], f32, isOutput=False)
    vT = nc.declare_dram_parameter("vT", [BC, L, V], bf16, isOutput=False)
    mT = nc.declare_dram_parameter("mT", [BC, L], i32, isOutput=False)
    qT = nc.declare_dram_parameter("qT", [Q, BC], f32, isOutput=False)
    Wr = nc.declare_dram_parameter("Wr", [Q, KW * C], f32, isOutput=False)
    Br = nc.declare_dram_parameter("Br", [128, M12], f32, isOutput=False)
    a_out = nc.declare_dram_parameter("a_out", [BC, L], f32, isOutput=True)
    e_out = nc.declare_dram_parameter("e_out", [BC, L], f32, isOutput=True)
    t_out = nc.declare_dram_parameter("t_out", [BC, V], f32, isOutput=True)

    with ExitStack() as ctx:
        tc = ctx.enter_context(tile.TileContext(nc))
        singles = ctx.enter_context(tc.tile_pool(name="singles", bufs=1))
        sa = ctx.enter_context(tc.tile_pool(name="sa", bufs=2))
        big = ctx.enter_context(tc.tile_pool(name="big", bufs=1))
        kpool = ctx.enter_context(tc.tile_pool(name="kpool", bufs=6))
        vpool = ctx.enter_context(tc.tile_pool(name="vpool", bufs=9))
        pq = ctx.enter_context(tc.tile_pool(name="pq", bufs=1, space="PSUM"))
        pcv = ctx.enter_context(tc.tile_pool(name="pcv", bufs=4, space="PSUM"))
        ptr = ctx.enter_context(tc.tile_pool(name="ptr", bufs=1, space="PSUM"))
        pat = ctx.enter_context(tc.tile_pool(name="pat", bufs=2, space="PSUM"))

        # ---- replicated params / small tensors ----
        wr_sb = singles.tile([128, Q // 128, KW * C], f32, tag="wr")
        q_sb = singles.tile([128, Q // 128, BC], f32, tag="q")
        b_sb = singles.tile([128, M12], f32, tag="bias")
        kern = singles.tile([128, M12, BC], f32, tag="kern")
        ident = singles.tile([128, 128], f32, tag="ident")
        maskf = singles.tile([BC, L], f32, tag="maskf")
        A_sb = singles.tile([BC, L], f32, tag="a")
        ET = singles.tile([128, L // 128, BC], bf16, tag="et")
        nmx = singles.tile([BC, 1], f32, tag="nmx")
        ssum = singles.tile([BC, 1], f32, tag="ssum")
        sinv = singles.tile([BC, 1], f32, tag="sinv")
        # mask-int tile borrows a k-pool slot (same byte size, freed after cast)
        msk_i = kpool.tile([BC, L], i32, tag="k")

        nc.sync.dma_start(out=wr_sb, in_=Wr[:].rearrange("(qc p) n -> p qc n", p=128))
        nc.sync.dma_start(out=q_sb, in_=qT[:].rearrange("(qc p) b -> p qc b", p=128))
        nc.sync.dma_start(out=b_sb, in_=Br[:])
        nc.sync.dma_start(out=msk_i, in_=mT[:])
        make_identity(nc, ident)

        # mask -> f32; additive mask: (m-1)*NEGBIG in {0, -NEGBIG} (in place)
        nc.vector.tensor_copy(out=maskf, in_=msk_i)
        nc.vector.tensor_scalar(
            out=maskf, in0=maskf, scalar1=-1.0, scalar2=NEGBIG,
            op0=mybir.AluOpType.add, op1=mybir.AluOpType.mult,
        )

        # ---- kern[p, m, b] = (q @ W.T + b) in (tap, channel) order ----
        for m in range(M12):
            pqt = pq.tile([128, BC], f32, tag="pq")
            for qc in range(Q // 128):
                nc.tensor.matmul(
                    pqt,
                    wr_sb[:, qc, m * 128:(m + 1) * 128],
                    q_sb[:, qc, :],
                    start=(qc == 0), stop=(qc == Q // 128 - 1),
                )
            nc.vector.tensor_scalar_add(
                out=kern[:, m, :], in0=pqt, scalar1=b_sb[:, m:m + 1]
            )

        # ---- t_w[l, b] = sum_c k[l, b, c] * kern[b, c, w]  (M=3: one rhs
        # pass computes all three taps). Taps land via DMA partition-scatter
        # into Sk8[b] = [3, L+3] rows (t_w[j] at (w, 1+j)); a single skewed
        # strided-AP reduce then forms a[l] = t0[l-1] + t1[l] + t2[l+1].
        Sk8 = big.tile([BC, KW, L + 3], f32, tag="sk8")
        nc.vector.memset(Sk8[:, 0, 0:1], 0.0)          # t0[-1] = 0
        nc.vector.memset(Sk8[:, 2, L + 1:L + 2], 0.0)  # t2[L]  = 0
        kern_r = kern.rearrange("p (w cc) b -> p cc w b", w=KW)
        for b_ in range(BC):
            ksb = []
            for cc in range(C // 128):
                kt = kpool.tile([128, L], f32, tag="k")
                nc.sync.dma_start(
                    out=kt, in_=kT[b_, cc * 128:(cc + 1) * 128, :]
                )
                ksb.append(kt)
            scv = sa.tile([KW, L], f32, tag="scv")
            for lc in range(L // 512):
                cv = pcv.tile([KW, 512], f32, tag="cv")
                for cc in range(C // 128):
                    nc.tensor.matmul(
                        cv,
                        kern_r[:, cc, :, b_],
                        ksb[cc][:, lc * 512:(lc + 1) * 512],
                        start=(cc == 0), stop=(cc == C // 128 - 1),
                    )
                nc.scalar.copy(out=scv[:, lc * 512:(lc + 1) * 512], in_=cv)
            # partition-scatter: [3, L] rows -> single partition b_ (DMA only)
            nc.sync.dma_start(out=Sk8[b_:b_ + 1, :, 1:L + 1], in_=scv)

        # skewed 3-tap reduce: addr(l, w) = w*(L+4) + l over Sk8's [3, L+3]
        # row-major free space hits exactly t_w[l-1+w]
        sk_full = Sk8[:]
        skew = bass.AP(
            tensor=sk_full.tensor,
            offset=sk_full.offset,
            ap=[sk_full.ap[0], [1, L], [L + 4, KW]],
        )
        nc.vector.tensor_reduce(
            out=A_sb, in_=skew, op=mybir.AluOpType.add, axis=mybir.AxisListType.X,
        )

        # ---- masked softmax over l (b on partitions 0..7) ----
        # Am reuses Sk8's slot (released by the reduce); becomes e_ij in place
        Am = big.tile([BC, L], f32, tag="sk8")
        E_sb = Am
        nc.vector.tensor_add(out=Am, in0=A_sb, in1=maskf)
        nc.vector.tensor_reduce(
            out=nmx, in_=Am, op=mybir.AluOpType.max,
            axis=mybir.AxisListType.X, negate=True,
        )
        nc.scalar.activation(
            out=E_sb, in_=Am, func=mybir.ActivationFunctionType.Exp,
            bias=nmx[:, 0:1], scale=1.0, accum_out=ssum,
        )
        nc.vector.reciprocal(out=sinv, in_=ssum)
        nc.vector.tensor_scalar_mul(out=E_sb, in0=E_sb, scalar1=sinv[:, 0:1])

        nc.sync.dma_start(out=a_out[:], in_=A_sb)
        nc.sync.dma_start(out=e_out[:], in_=E_sb)

        # ---- ET[p, lt, b] = E[b, lt*128+p] (PE transpose, cast to bf16) ----
        for lt in range(L // 128):
            trp = ptr.tile([128, BC], f32, tag="tr")
            nc.tensor.transpose(trp, E_sb[:, lt * 128:(lt + 1) * 128], ident[0:BC, 0:BC])
            nc.vector.tensor_copy(out=ET[:, lt, :], in_=trp)

        # ---- attend[b, :] = sum_l e[l, b] * v[l, b, :] ----
        # lhsT = ET[:, lt, :] gives an [8, 512] output whose row m pairs
        # e(:, m) with v(:, b); only row b is real — DMA just that row out.
        for b_ in range(BC):
            vsb = []
            for jj in range(2):
                vt = vpool.tile([128, 8, V], bf16, tag="v")
                nc.sync.dma_start(
                    out=vt,
                    in_=vT[b_, jj * 1024:(jj + 1) * 1024, :].rearrange(
                        "(i p) v -> p i v", p=128
                    ),
                )
                vsb.append(vt)
            att_ps = pat.tile([BC, V], f32, tag="atp")
            for lt in range(L // 128):
                nc.tensor.matmul(
                    att_ps,
                    ET[:, lt, :],
                    vsb[lt // 8][:, lt % 8, :],
                    start=(lt == 0), stop=(lt == L // 128 - 1),
                )
            att_sb = sa.tile([BC, V], f32, tag="attsb")
            nc.vector.tensor_copy(out=att_sb, in_=att_ps)
            nc.sync.dma_start(out=t_out[b_:b_ + 1, :], in_=att_sb[b_:b_ + 1, :])

    nc.compile()
    return nc


def get_nc():
    global _NC
    if _NC is None:
        _NC = _build()
    return _NC


def make_in_maps(q, k, v, k_mask, W, b):
    import ml_dtypes

    q = np.ascontiguousarray(np.asarray(q, dtype=np.float32))
    k = np.asarray(k, dtype=np.float32)
    v = np.asarray(v, dtype=np.float32)
    k_mask = np.asarray(k_mask, dtype=np.int32)
    W = np.asarray(W, dtype=np.float32)
    b = np.asarray(b, dtype=np.float32)

    # W column reorder: Wr[q, w*C + c] = W[c*KW + w, q]
    Wr = np.ascontiguousarray(W.reshape(C, KW, Q).transpose(2, 1, 0).reshape(Q, KW * C))
    Br = np.ascontiguousarray(b.reshape(C, KW).T.reshape(KW * C).reshape(M12, 128).T)

    in_maps = []
    for i in range(NCORES):
        sl = slice(i * BC, (i + 1) * BC)
        in_maps.append({
            "kT": np.ascontiguousarray(k[:, sl, :].transpose(1, 2, 0)),
            "vT": np.ascontiguousarray(v[:, sl, :].transpose(1, 0, 2)).astype(
                ml_dtypes.bfloat16
            ),
            "mT": np.ascontiguousarray(k_mask[:, sl].T),
            "qT": np.ascontiguousarray(q[sl, :].T),
            "Wr": Wr,
            "Br": Br,
        })
    return in_maps


def assemble(results):
    a = np.concatenate([r["a_out"] for r in results], axis=0).T.copy()
    e = np.concatenate([r["e_out"] for r in results], axis=0).T.copy()
    att = np.concatenate([r["t_out"] for r in results], axis=0)
    return (
        np.ascontiguousarray(a, dtype=np.float32),
        np.ascontiguousarray(e, dtype=np.float32),
        np.ascontiguousarray(att, dtype=np.float32),
    )


def kernel(q, k, v, k_mask, W, b):
    from concourse.bass_utils import run_bass_kernel_spmd

    nc = get_nc()
    in_maps = make_in_maps(q, k, v, k_mask, W, b)
    res = run_bass_kernel_spmd(nc, in_maps, core_ids=list(range(NCORES)))
    return assemble(res.results)


# revision 22
# speedup vs baseline: 2.1618x; 1.3327x over previous
"""Trainium2 Bass kernel for nn_AttCNN4Weight (sparse_attention).

Data-parallel over batch: each of the 8 NeuronCores handles 8 of the 64
batch elements end-to-end (dynamic per-sample conv kernel -> sliding-window
score -> masked softmax over kv_len -> weighted sum of v). No collectives.

Host-side work is layout only: batch sharding, transposes so every DMA
moves multi-KB contiguous rows, a column reorder of W to (tap, channel)
order, and a bf16 cast of v (the attend reduction tolerates bf16 easily;
halves v HBM traffic).

Key trick: the KW=3 sliding-window score is a single matmul contraction
over (tap, channel) = 1536, where each tap's k-operand is just a +/-1
shifted free-dim slice of the same padded k tile. That keeps every matmul
output at PSUM partition base 0 (a hardware requirement) with M=1.
"""

import sys

if "/opt/trn_rl_repo" not in sys.path:
    sys.path.insert(0, "/opt/trn_rl_repo")

import numpy as np
from contextlib import ExitStack

L, B, C, Q, V, KW = 2048, 64, 512, 512, 512, 3
NCORES = 8
BC = B // NCORES          # 8 batch elements per core
M12 = KW * (C // 128)     # 12 contraction chunks of (tap, channel)
NEGBIG = 3.0e38           # additive mask constant (finite, exp() underflows to 0)

_NC = None


def _build():
    import concourse.bass as bass
    import concourse.bacc as bacc
    import concourse.tile as tile
    from concourse import mybir
    from concourse.masks import make_identity

    f32 = mybir.dt.float32
    f32r = mybir.dt.float32r
    bf16 = mybir.dt.bfloat16
    i32 = mybir.dt.int32

    nc = bacc.Bacc(None)

    kT = nc.declare_dram_parameter("kT", [BC, C, L], f32, isOutput=False)
    vT = nc.declare_dram_parameter("vT", [BC, L, V], bf16, isOutput=False)
    mT = nc.declare_dram_parameter("mT", [BC, L], i32, isOutput=False)
    qT = nc.declare_dram_parameter("qT", [Q, BC], f32, isOutput=False)
    Wr = nc.declare_dram_parameter("Wr", [Q, KW * C], f32, isOutput=False)
    Br = nc.declare_dram_parameter("Br", [128, M12], f32, isOutput=False)
    a_out = nc.declare_dram_parameter("a_out", [BC, L], f32, isOutput=True)
    e_out = nc.declare_dram_parameter("e_out", [BC, L], f32, isOutput=True)
    t_out = nc.declare_dram_parameter("t_out", [BC, V], f32, isOutput=True)

    with ExitStack() as ctx:
        tc = ctx.enter_context(tile.TileContext(nc))
        singles = ctx.enter_context(tc.tile_pool(name="singles", bufs=1))
        sa = ctx.enter_context(tc.tile_pool(name="sa", bufs=2))
        big = ctx.enter_context(tc.tile_pool(name="big", bufs=1))
        kpool = ctx.enter_context(tc.tile_pool(name="kpool", bufs=6))
        vpool = ctx.enter_context(tc.tile_pool(name="vpool", bufs=9))
        pq = ctx.enter_context(tc.tile_pool(name="pq", bufs=1, space="PSUM"))
        pcv = ctx.enter_context(tc.tile_pool(name="pcv", bufs=4, space="PSUM"))
        ptr = ctx.enter_context(tc.tile_pool(name="ptr", bufs=1, space="PSUM"))
        pat = ctx.enter_context(tc.tile_pool(name="pat", bufs=2, space="PSUM"))

        # ---- replicated params / small tensors ----
        # f32r: single-pass fp32 matmul mode (fp32 is 2 half-rate passes);
        # producers must emit the f32r-rounded form, so tiles are typed f32r
        # and the f32 DRAM source is bitcast on the way in.
        wr_sb = singles.tile([128, Q // 128, KW * C], f32r, tag="wr")
        q_sb = singles.tile([128, Q // 128, BC], f32r, tag="q")
        b_sb = singles.tile([128, M12], f32, tag="bias")
        kern = singles.tile([128, M12, BC], f32r, tag="kern")
        ident = singles.tile([128, 128], f32, tag="ident")
        maskf = singles.tile([BC, L], f32, tag="maskf")
        A_sb = singles.tile([BC, L], f32, tag="a")
        ET = singles.tile([128, L // 128, BC], bf16, tag="et")
        nmx = singles.tile([BC, 1], f32, tag="nmx")
        ssum = singles.tile([BC, 1], f32, tag="ssum")
        sinv = singles.tile([BC, 1], f32, tag="sinv")
        # mask-int tile borrows a k-pool slot (same byte size, freed after cast)
        msk_i = kpool.tile([BC, L], i32, tag="k")

        nc.sync.dma_start(
            out=wr_sb, in_=Wr[:].rearrange("(qc p) n -> p qc n", p=128).bitcast(f32r)
        )
        nc.sync.dma_start(
            out=q_sb, in_=qT[:].rearrange("(qc p) b -> p qc b", p=128).bitcast(f32r)
        )
        nc.sync.dma_start(out=b_sb, in_=Br[:])
        nc.sync.dma_start(out=msk_i, in_=mT[:])
        make_identity(nc, ident)

        # mask -> f32; additive mask: (m-1)*NEGBIG in {0, -NEGBIG} (in place)
        nc.vector.tensor_copy(out=maskf, in_=msk_i)
        nc.vector.tensor_scalar(
            out=maskf, in0=maskf, scalar1=-1.0, scalar2=NEGBIG,
            op0=mybir.AluOpType.add, op1=mybir.AluOpType.mult,
        )

        # ---- kern[p, m, b] = (q @ W.T + b) in (tap, channel) order ----
        for m in range(M12):
            pqt = pq.tile([128, BC], f32, tag="pq")
            for qc in range(Q // 128):
                nc.tensor.matmul(
                    pqt,
                    wr_sb[:, qc, m * 128:(m + 1) * 128],
                    q_sb[:, qc, :],
                    start=(qc == 0), stop=(qc == Q // 128 - 1),
                )
            nc.vector.tensor_scalar_add(
                out=kern[:, m, :], in0=pqt, scalar1=b_sb[:, m:m + 1]
            )

        # ---- t_w[l, b] = sum_c k[l, b, c] * kern[b, c, w]  (M=3: one rhs
        # pass computes all three taps). Taps land via DMA partition-scatter
        # into Sk8[b] = [3, L+3] rows (t_w[j] at (w, 1+j)); a single skewed
        # strided-AP reduce then forms a[l] = t0[l-1] + t1[l] + t2[l+1].
        Sk8 = big.tile([BC, KW, L + 3], f32, tag="sk8")
        nc.vector.memset(Sk8[:, 0, 0:1], 0.0)          # t0[-1] = 0
        nc.vector.memset(Sk8[:, 2, L + 1:L + 2], 0.0)  # t2[L]  = 0
        kern_r = kern.rearrange("p (w cc) b -> p cc w b", w=KW)
        for b_ in range(BC):
            ksb = []
            for cc in range(C // 128):
                kt = kpool.tile([128, L], f32r, tag="k")
                nc.sync.dma_start(
                    out=kt, in_=kT[b_, cc * 128:(cc + 1) * 128, :].bitcast(f32r)
                )
                ksb.append(kt)
            scv = sa.tile([KW, L], f32, tag="scv")
            for lc in range(L // 512):
                cv = pcv.tile([KW, 512], f32, tag="cv")
                for cc in range(C // 128):
                    nc.tensor.matmul(
                        cv,
                        kern_r[:, cc, :, b_],
                        ksb[cc][:, lc * 512:(lc + 1) * 512],
                        start=(cc == 0), stop=(cc == C // 128 - 1),
                    )
                nc.scalar.copy(out=scv[:, lc * 512:(lc + 1) * 512], in_=cv)
            # partition-scatter: [3, L] rows -> single partition b_ (DMA only)
            nc.sync.dma_start(out=Sk8[b_:b_ + 1, :, 1:L + 1], in_=scv)

        # skewed 3-tap reduce: addr(l, w) = w*(L+4) + l over Sk8's [3, L+3]
        # row-major free space hits exactly t_w[l-1+w]
        sk_full = Sk8[:]
        skew = bass.AP(
            tensor=sk_full.tensor,
            offset=sk_full.offset,
            ap=[sk_full.ap[0], [1, L], [L + 4, KW]],
        )
        nc.vector.tensor_reduce(
            out=A_sb, in_=skew, op=mybir.AluOpType.add, axis=mybir.AxisListType.X,
        )

        # ---- masked softmax over l (b on partitions 0..7) ----
        # Am reuses Sk8's slot (released by the reduce); becomes e_ij in place
        Am = big.tile([BC, L], f32, tag="sk8")
        E_sb = Am
        nc.vector.tensor_add(out=Am, in0=A_sb, in1=maskf)
        nc.vector.tensor_reduce(
            out=nmx, in_=Am, op=mybir.AluOpType.max,
            axis=mybir.AxisListType.X, negate=True,
        )
        nc.scalar.activation(
            out=E_sb, in_=Am, func=mybir.ActivationFunctionType.Exp,
            bias=nmx[:, 0:1], scale=1.0, accum_out=ssum,
        )
        nc.vector.reciprocal(out=sinv, in_=ssum)
        nc.vector.tensor_scalar_mul(out=E_sb, in0=E_sb, scalar1=sinv[:, 0:1])

        nc.sync.dma_start(out=a_out[:], in_=A_sb)
        nc.sync.dma_start(out=e_out[:], in_=E_sb)

        # ---- ET[p, lt, b] = E[b, lt*128+p] (PE transpose, cast to bf16) ----
        for lt in range(L // 128):
            trp = ptr.tile([128, BC], f32, tag="tr")
            nc.tensor.transpose(trp, E_sb[:, lt * 128:(lt + 1) * 128], ident[0:BC, 0:BC])
            nc.vector.tensor_copy(out=ET[:, lt, :], in_=trp)

        # ---- attend[b, :] = sum_l e[l, b] * v[l, b, :] ----
        # lhsT = ET[:, lt, :] gives an [8, 512] output whose row m pairs
        # e(:, m) with v(:, b); only row b is real — DMA just that row out.
        for b_ in range(BC):
            vsb = []
            for jj in range(2):
                vt = vpool.tile([128, 8, V], bf16, tag="v")
                nc.sync.dma_start(
                    out=vt,
                    in_=vT[b_, jj * 1024:(jj + 1) * 1024, :].rearrange(
                        "(i p) v -> p i v", p=128
                    ),
                )
                vsb.append(vt)
            att_ps = pat.tile([BC, V], f32, tag="atp")
            for lt in range(L // 128):
                nc.tensor.matmul(
                    att_ps,
                    ET[:, lt, :],
                    vsb[lt // 8][:, lt % 8, :],
                    start=(lt == 0), stop=(lt == L // 128 - 1),
                )
            att_sb = sa.tile([BC, V], f32, tag="attsb")
            nc.vector.tensor_copy(out=att_sb, in_=att_ps)
            nc.sync.dma_start(out=t_out[b_:b_ + 1, :], in_=att_sb[b_:b_ + 1, :])

    nc.compile()
    return nc


def get_nc():
    global _NC
    if _NC is None:
        _NC = _build()
    return _NC


def make_in_maps(q, k, v, k_mask, W, b):
    import ml_dtypes

    q = np.ascontiguousarray(np.asarray(q, dtype=np.float32))
    k = np.asarray(k, dtype=np.float32)
    v = np.asarray(v, dtype=np.float32)
    k_mask = np.asarray(k_mask, dtype=np.int32)
    W = np.asarray(W, dtype=np.float32)
    b = np.asarray(b, dtype=np.float32)

    # W column reorder: Wr[q, w*C + c] = W[c*KW + w, q]
    Wr = np.ascontiguousarray(W.reshape(C, KW, Q).transpose(2, 1, 0).reshape(Q, KW * C))
    Br = np.ascontiguousarray(b.reshape(C, KW).T.reshape(KW * C).reshape(M12, 128).T)

    in_maps = []
    for i in range(NCORES):
        sl = slice(i * BC, (i + 1) * BC)
        in_maps.append({
            "kT": np.ascontiguousarray(k[:, sl, :].transpose(1, 2, 0)),
            "vT": np.ascontiguousarray(v[:, sl, :].transpose(1, 0, 2)).astype(
                ml_dtypes.bfloat16
            ),
            "mT": np.ascontiguousarray(k_mask[:, sl].T),
            "qT": np.ascontiguousarray(q[sl, :].T),
            "Wr": Wr,
            "Br": Br,
        })
    return in_maps


def assemble(results):
    a = np.concatenate([r["a_out"] for r in results], axis=0).T.copy()
    e = np.concatenate([r["e_out"] for r in results], axis=0).T.copy()
    att = np.concatenate([r["t_out"] for r in results], axis=0)
    return (
        np.ascontiguousarray(a, dtype=np.float32),
        np.ascontiguousarray(e, dtype=np.float32),
        np.ascontiguousarray(att, dtype=np.float32),
    )


def kernel(q, k, v, k_mask, W, b):
    from concourse.bass_utils import run_bass_kernel_spmd

    nc = get_nc()
    in_maps = make_in_maps(q, k, v, k_mask, W, b)
    res = run_bass_kernel_spmd(nc, in_maps, core_ids=list(range(NCORES)))
    return assemble(res.results)


# revision 28
# speedup vs baseline: 2.2785x; 1.0540x over previous
"""Trainium2 Bass kernel for nn_AttCNN4Weight (sparse_attention).

Data-parallel over batch: each of the 8 NeuronCores handles 8 of the 64
batch elements end-to-end (dynamic per-sample conv kernel -> sliding-window
score -> masked softmax over kv_len -> weighted sum of v). No collectives.

Host-side work is layout only: batch sharding, transposes so every DMA
moves multi-KB contiguous rows, a column reorder of W to (tap, channel)
order, and a bf16 cast of v (the attend reduction tolerates bf16 easily;
halves v HBM traffic).

Key trick: the KW=3 sliding-window score is a single matmul contraction
over (tap, channel) = 1536, where each tap's k-operand is just a +/-1
shifted free-dim slice of the same padded k tile. That keeps every matmul
output at PSUM partition base 0 (a hardware requirement) with M=1.
"""

import sys

if "/opt/trn_rl_repo" not in sys.path:
    sys.path.insert(0, "/opt/trn_rl_repo")

import numpy as np
from contextlib import ExitStack

L, B, C, Q, V, KW = 2048, 64, 512, 512, 512, 3
NCORES = 8
BC = B // NCORES          # 8 batch elements per core
M12 = KW * (C // 128)     # 12 contraction chunks of (tap, channel)
NEGBIG = 3.0e38           # additive mask constant (finite, exp() underflows to 0)

_NC = None


def _build():
    import concourse.bass as bass
    import concourse.bacc as bacc
    import concourse.tile as tile
    from concourse import mybir
    from concourse.masks import make_identity

    f32 = mybir.dt.float32
    f32r = mybir.dt.float32r
    bf16 = mybir.dt.bfloat16
    i32 = mybir.dt.int32

    nc = bacc.Bacc(None)

    kT = nc.declare_dram_parameter("kT", [BC, C, L], f32, isOutput=False)
    vT = nc.declare_dram_parameter("vT", [BC, L, V], bf16, isOutput=False)
    mT = nc.declare_dram_parameter("mT", [BC, L], i32, isOutput=False)
    qT = nc.declare_dram_parameter("qT", [Q, BC], f32, isOutput=False)
    Wr = nc.declare_dram_parameter("Wr", [Q, KW * C], f32, isOutput=False)
    Br = nc.declare_dram_parameter("Br", [128, M12], f32, isOutput=False)
    a_out = nc.declare_dram_parameter("a_out", [BC, L], f32, isOutput=True)
    e_out = nc.declare_dram_parameter("e_out", [BC, L], f32, isOutput=True)
    t_out = nc.declare_dram_parameter("t_out", [BC, V], f32, isOutput=True)

    with ExitStack() as ctx:
        tc = ctx.enter_context(tile.TileContext(nc))
        singles = ctx.enter_context(tc.tile_pool(name="singles", bufs=1))
        sa = ctx.enter_context(tc.tile_pool(name="sa", bufs=2))
        big = ctx.enter_context(tc.tile_pool(name="big", bufs=1))
        kpool = ctx.enter_context(tc.tile_pool(name="kpool", bufs=6))
        vpool = ctx.enter_context(tc.tile_pool(name="vpool", bufs=9))
        pq = ctx.enter_context(tc.tile_pool(name="pq", bufs=1, space="PSUM"))
        pcv = ctx.enter_context(tc.tile_pool(name="pcv", bufs=4, space="PSUM"))
        ptr = ctx.enter_context(tc.tile_pool(name="ptr", bufs=1, space="PSUM"))
        pat = ctx.enter_context(tc.tile_pool(name="pat", bufs=2, space="PSUM"))

        # ---- replicated params / small tensors ----
        # f32r: single-pass fp32 matmul mode (fp32 is 2 half-rate passes);
        # producers must emit the f32r-rounded form, so tiles are typed f32r
        # and the f32 DRAM source is bitcast on the way in.
        wr_sb = singles.tile([128, Q // 128, KW * C], f32r, tag="wr")
        q_sb = singles.tile([128, Q // 128, BC], f32r, tag="q")
        b_sb = singles.tile([128, M12], f32, tag="bias")
        kern = singles.tile([128, M12, BC], f32r, tag="kern")
        ident = singles.tile([128, 128], f32, tag="ident")
        maskf = singles.tile([BC, L], f32, tag="maskf")
        A_sb = singles.tile([BC, L], f32, tag="a")
        ET = singles.tile([128, L // 128, BC], bf16, tag="et")
        nmx = singles.tile([BC, 1], f32, tag="nmx")
        ssum = singles.tile([BC, 1], f32, tag="ssum")
        sinv = singles.tile([BC, 1], f32, tag="sinv")
        # mask-int tile borrows a k-pool slot (same byte size, freed after cast)
        msk_i = kpool.tile([BC, L], i32, tag="k")

        nc.sync.dma_start(
            out=q_sb, in_=qT[:].rearrange("(qc p) b -> p qc b", p=128).bitcast(f32r)
        )
        nc.sync.dma_start(out=b_sb, in_=Br[:])
        # Wr arrives chunk-by-chunk so qW (and then conv) can start before
        # the full 3MB lands
        wr_src = Wr[:].rearrange("(qc p) n -> p qc n", p=128).bitcast(f32r)
        for m in range(M12):
            nc.sync.dma_start(
                out=wr_sb[:, :, m * 128:(m + 1) * 128],
                in_=wr_src[:, :, m * 128:(m + 1) * 128],
            )
        nc.sync.dma_start(out=msk_i, in_=mT[:])
        make_identity(nc, ident)

        # mask -> f32; additive mask: (m-1)*NEGBIG in {0, -NEGBIG} (in place)
        nc.vector.tensor_copy(out=maskf, in_=msk_i)
        nc.vector.tensor_scalar(
            out=maskf, in0=maskf, scalar1=-1.0, scalar2=NEGBIG,
            op0=mybir.AluOpType.add, op1=mybir.AluOpType.mult,
        )

        # ---- kern[p, m, b] = (q @ W.T + b) in (tap, channel) order ----
        for m in range(M12):
            pqt = pq.tile([128, BC], f32, tag="pq")
            for qc in range(Q // 128):
                nc.tensor.matmul(
                    pqt,
                    wr_sb[:, qc, m * 128:(m + 1) * 128],
                    q_sb[:, qc, :],
                    start=(qc == 0), stop=(qc == Q // 128 - 1),
                )
            nc.vector.tensor_scalar_add(
                out=kern[:, m, :], in0=pqt, scalar1=b_sb[:, m:m + 1]
            )

        # ---- t_w[l, b] = sum_c k[l, b, c] * kern[b, c, w]  (M=3: one rhs
        # pass computes all three taps). Taps land via DMA partition-scatter
        # into Sk8[b] = [3, L+3] rows (t_w[j] at (w, 1+j)); a single skewed
        # strided-AP reduce then forms a[l] = t0[l-1] + t1[l] + t2[l+1].
        Sk8 = big.tile([BC, KW, L + 3], f32, tag="sk8")
        nc.vector.memset(Sk8[:, 0, 0:1], 0.0)          # t0[-1] = 0
        nc.vector.memset(Sk8[:, 2, L + 1:L + 2], 0.0)  # t2[L]  = 0
        kern_r = kern.rearrange("p (w cc) b -> p cc w b", w=KW)
        for b_ in range(BC):
            ksb = []
            for cc in range(C // 128):
                kt = kpool.tile([128, L], f32r, tag="k")
                nc.sync.dma_start(
                    out=kt, in_=kT[b_, cc * 128:(cc + 1) * 128, :].bitcast(f32r)
                )
                ksb.append(kt)
            scv = sa.tile([KW, L], f32, tag="scv")
            for lc in range(L // 512):
                cv = pcv.tile([KW, 512], f32, tag="cv")
                for cc in range(C // 128):
                    nc.tensor.matmul(
                        cv,
                        kern_r[:, cc, :, b_],
                        ksb[cc][:, lc * 512:(lc + 1) * 512],
                        start=(cc == 0), stop=(cc == C // 128 - 1),
                    )
                nc.scalar.copy(out=scv[:, lc * 512:(lc + 1) * 512], in_=cv)
            # partition-scatter: [3, L] rows -> single partition b_. Issued
            # from the scalar engine's HWDGE ring so the sync ring (k/v input
            # streams) never head-of-line blocks on compute-dependent stores.
            nc.scalar.dma_start(out=Sk8[b_:b_ + 1, :, 1:L + 1], in_=scv)

        # skewed 3-tap reduce: addr(l, w) = w*(L+4) + l over Sk8's [3, L+3]
        # row-major free space hits exactly t_w[l-1+w]
        sk_full = Sk8[:]
        skew = bass.AP(
            tensor=sk_full.tensor,
            offset=sk_full.offset,
            ap=[sk_full.ap[0], [1, L], [L + 4, KW]],
        )
        nc.vector.tensor_reduce(
            out=A_sb, in_=skew, op=mybir.AluOpType.add, axis=mybir.AxisListType.X,
        )

        # ---- masked softmax over l (b on partitions 0..7) ----
        # Am reuses Sk8's slot (released by the reduce); becomes e_ij in place
        Am = big.tile([BC, L], f32, tag="sk8")
        E_sb = Am
        nc.vector.tensor_add(out=Am, in0=A_sb, in1=maskf)
        nc.vector.tensor_reduce(
            out=nmx, in_=Am, op=mybir.AluOpType.max,
            axis=mybir.AxisListType.X, negate=True,
        )
        nc.scalar.activation(
            out=E_sb, in_=Am, func=mybir.ActivationFunctionType.Exp,
            bias=nmx[:, 0:1], scale=1.0, accum_out=ssum,
        )
        nc.vector.reciprocal(out=sinv, in_=ssum)
        nc.vector.tensor_scalar_mul(out=E_sb, in0=E_sb, scalar1=sinv[:, 0:1])

        nc.scalar.dma_start(out=a_out[:], in_=A_sb)
        nc.scalar.dma_start(out=e_out[:], in_=E_sb)

        # ---- ET[p, lt, b] = E[b, lt*128+p] (PE transpose, cast to bf16) ----
        for lt in range(L // 128):
            trp = ptr.tile([128, BC], f32, tag="tr")
            nc.tensor.transpose(trp, E_sb[:, lt * 128:(lt + 1) * 128], ident[0:BC, 0:BC])
            nc.vector.tensor_copy(out=ET[:, lt, :], in_=trp)

        # ---- attend[b, :] = sum_l e[l, b] * v[l, b, :] ----
        # lhsT = ET[:, lt, :] gives an [8, 512] output whose row m pairs
        # e(:, m) with v(:, b); only row b is real — DMA just that row out.
        for b_ in range(BC):
            vsb = []
            for jj in range(2):
                vt = vpool.tile([128, 8, V], bf16, tag="v")
                # on the sync ring AFTER the whole k stream: v intentionally
                # does not compete with k for bandwidth (conv->softmax is the
                # serial prefix; prefetching v early would delay it), and with
                # outputs moved to the scalar ring nothing blocks v during
                # the softmax window
                nc.sync.dma_start(
                    out=vt,
                    in_=vT[b_, jj * 1024:(jj + 1) * 1024, :].rearrange(
                        "(i p) v -> p i v", p=128
                    ),
                )
                vsb.append(vt)
            att_ps = pat.tile([BC, V], f32, tag="atp")
            for lt in range(L // 128):
                nc.tensor.matmul(
                    att_ps,
                    ET[:, lt, :],
                    vsb[lt // 8][:, lt % 8, :],
                    start=(lt == 0), stop=(lt == L // 128 - 1),
                )
            att_sb = sa.tile([BC, V], f32, tag="attsb")
            nc.vector.tensor_copy(out=att_sb, in_=att_ps)
            nc.scalar.dma_start(out=t_out[b_:b_ + 1, :], in_=att_sb[b_:b_ + 1, :])

    nc.compile()
    return nc


def get_nc():
    global _NC
    if _NC is None:
        _NC = _build()
    return _NC


def make_in_maps(q, k, v, k_mask, W, b):
    import ml_dtypes

    q = np.ascontiguousarray(np.asarray(q, dtype=np.float32))
    k = np.asarray(k, dtype=np.float32)
    v = np.asarray(v, dtype=np.float32)
    k_mask = np.asarray(k_mask, dtype=np.int32)
    W = np.asarray(W, dtype=np.float32)
    b = np.asarray(b, dtype=np.float32)

    # W column reorder: Wr[q, w*C + c] = W[c*KW + w, q]
    Wr = np.ascontiguousarray(W.reshape(C, KW, Q).transpose(2, 1, 0).reshape(Q, KW * C))
    Br = np.ascontiguousarray(b.reshape(C, KW).T.reshape(KW * C).reshape(M12, 128).T)

    in_maps = []
    for i in range(NCORES):
        sl = slice(i * BC, (i + 1) * BC)
        in_maps.append({
            "kT": np.ascontiguousarray(k[:, sl, :].transpose(1, 2, 0)),
            "vT": np.ascontiguousarray(v[:, sl, :].transpose(1, 0, 2)).astype(
                ml_dtypes.bfloat16
            ),
            "mT": np.ascontiguousarray(k_mask[:, sl].T),
            "qT": np.ascontiguousarray(q[sl, :].T),
            "Wr": Wr,
            "Br": Br,
        })
    return in_maps


def assemble(results):
    a = np.concatenate([r["a_out"] for r in results], axis=0).T.copy()
    e = np.concatenate([r["e_out"] for r in results], axis=0).T.copy()
    att = np.concatenate([r["t_out"] for r in results], axis=0)
    return (
        np.ascontiguousarray(a, dtype=np.float32),
        np.ascontiguousarray(e, dtype=np.float32),
        np.ascontiguousarray(att, dtype=np.float32),
    )


def kernel(q, k, v, k_mask, W, b):
    from concourse.bass_utils import run_bass_kernel_spmd

    nc = get_nc()
    in_maps = make_in_maps(q, k, v, k_mask, W, b)
    res = run_bass_kernel_spmd(nc, in_maps, core_ids=list(range(NCORES)))
    return assemble(res.results)


# revision 31
# speedup vs baseline: 2.3001x; 1.0095x over previous
"""Trainium2 Bass kernel for nn_AttCNN4Weight (sparse_attention).

Data-parallel over batch: each of the 8 NeuronCores handles 8 of the 64
batch elements end-to-end (dynamic per-sample conv kernel -> sliding-window
score -> masked softmax over kv_len -> weighted sum of v). No collectives.

Host-side work is layout only: batch sharding, transposes so every DMA
moves multi-KB contiguous rows, a column reorder of W to (tap, channel)
order, and a bf16 cast of v (the attend reduction tolerates bf16 easily;
halves v HBM traffic).

Key trick: the KW=3 sliding-window score is a single matmul contraction
over (tap, channel) = 1536, where each tap's k-operand is just a +/-1
shifted free-dim slice of the same padded k tile. That keeps every matmul
output at PSUM partition base 0 (a hardware requirement) with M=1.
"""

import sys

if "/opt/trn_rl_repo" not in sys.path:
    sys.path.insert(0, "/opt/trn_rl_repo")

import numpy as np
from contextlib import ExitStack

L, B, C, Q, V, KW = 2048, 64, 512, 512, 512, 3
NCORES = 8
BC = B // NCORES          # 8 batch elements per core
M12 = KW * (C // 128)     # 12 contraction chunks of (tap, channel)
NEGBIG = 3.0e38           # additive mask constant (finite, exp() underflows to 0)

_NC = None


def _build():
    import concourse.bass as bass
    import concourse.bacc as bacc
    import concourse.tile as tile
    from concourse import mybir
    from concourse.masks import make_identity

    f32 = mybir.dt.float32
    f32r = mybir.dt.float32r
    bf16 = mybir.dt.bfloat16
    i32 = mybir.dt.int32

    nc = bacc.Bacc(None)

    kT = nc.declare_dram_parameter("kT", [BC, C, L], f32, isOutput=False)
    vT = nc.declare_dram_parameter("vT", [BC, L, V], bf16, isOutput=False)
    mT = nc.declare_dram_parameter("mT", [BC, L], i32, isOutput=False)
    qT = nc.declare_dram_parameter("qT", [Q, BC], f32, isOutput=False)
    Wr = nc.declare_dram_parameter("Wr", [Q, KW * C], f32, isOutput=False)
    Br = nc.declare_dram_parameter("Br", [128, M12], f32, isOutput=False)
    a_out = nc.declare_dram_parameter("a_out", [BC, L], f32, isOutput=True)
    e_out = nc.declare_dram_parameter("e_out", [BC, L], f32, isOutput=True)
    t_out = nc.declare_dram_parameter("t_out", [BC, V], f32, isOutput=True)

    with ExitStack() as ctx:
        tc = ctx.enter_context(tile.TileContext(nc))
        singles = ctx.enter_context(tc.tile_pool(name="singles", bufs=1))
        sa = ctx.enter_context(tc.tile_pool(name="sa", bufs=2))
        big = ctx.enter_context(tc.tile_pool(name="big", bufs=1))
        kpool = ctx.enter_context(tc.tile_pool(name="kpool", bufs=6))
        vpool = ctx.enter_context(tc.tile_pool(name="vpool", bufs=12))
        pq = ctx.enter_context(tc.tile_pool(name="pq", bufs=1, space="PSUM"))
        pcv = ctx.enter_context(tc.tile_pool(name="pcv", bufs=4, space="PSUM"))
        ptr = ctx.enter_context(tc.tile_pool(name="ptr", bufs=1, space="PSUM"))
        pat = ctx.enter_context(tc.tile_pool(name="pat", bufs=2, space="PSUM"))

        # ---- replicated params / small tensors ----
        # f32r: single-pass fp32 matmul mode (fp32 is 2 half-rate passes);
        # producers must emit the f32r-rounded form, so tiles are typed f32r
        # and the f32 DRAM source is bitcast on the way in.
        # wr_sb, Sk8 and Am have disjoint lifetimes and share one 24KB slot.
        wr_sb = big.tile([128, Q // 128, KW * C], f32r, tag="sk8")
        q_sb = singles.tile([128, Q // 128, BC], f32r, tag="q")
        b_sb = singles.tile([128, M12], f32, tag="bias")
        kern = singles.tile([128, M12, BC], f32r, tag="kern")
        ident = singles.tile([128, 128], f32, tag="ident")
        maskf = singles.tile([BC, L], f32, tag="maskf")
        A_sb = singles.tile([BC, L], f32, tag="a")
        ET = singles.tile([128, L // 128, BC], bf16, tag="et")
        nmx = singles.tile([BC, 1], f32, tag="nmx")
        ssum = singles.tile([BC, 1], f32, tag="ssum")
        sinv = singles.tile([BC, 1], f32, tag="sinv")
        # mask-int tile borrows a k-pool slot (same byte size, freed after cast)
        msk_i = kpool.tile([BC, L], i32, tag="k")

        nc.sync.dma_start(
            out=q_sb, in_=qT[:].rearrange("(qc p) b -> p qc b", p=128).bitcast(f32r)
        )
        nc.sync.dma_start(out=b_sb, in_=Br[:])
        # Wr arrives chunk-by-chunk so qW (and then conv) can start before
        # the full 3MB lands
        wr_src = Wr[:].rearrange("(qc p) n -> p qc n", p=128).bitcast(f32r)
        for m in range(M12):
            nc.sync.dma_start(
                out=wr_sb[:, :, m * 128:(m + 1) * 128],
                in_=wr_src[:, :, m * 128:(m + 1) * 128],
            )
        nc.sync.dma_start(out=msk_i, in_=mT[:])
        make_identity(nc, ident)

        # mask -> f32; additive mask: (m-1)*NEGBIG in {0, -NEGBIG} (in place)
        nc.vector.tensor_copy(out=maskf, in_=msk_i)
        nc.vector.tensor_scalar(
            out=maskf, in0=maskf, scalar1=-1.0, scalar2=NEGBIG,
            op0=mybir.AluOpType.add, op1=mybir.AluOpType.mult,
        )

        # ---- kern[p, m, b] = (q @ W.T + b) in (tap, channel) order ----
        for m in range(M12):
            pqt = pq.tile([128, BC], f32, tag="pq")
            for qc in range(Q // 128):
                nc.tensor.matmul(
                    pqt,
                    wr_sb[:, qc, m * 128:(m + 1) * 128],
                    q_sb[:, qc, :],
                    start=(qc == 0), stop=(qc == Q // 128 - 1),
                )
            nc.vector.tensor_scalar_add(
                out=kern[:, m, :], in0=pqt, scalar1=b_sb[:, m:m + 1]
            )

        # ---- t_w[l, b] = sum_c k[l, b, c] * kern[b, c, w]  (M=3: one rhs
        # pass computes all three taps). Taps land via DMA partition-scatter
        # into Sk8[b] = [3, L+3] rows (t_w[j] at (w, 1+j)); a single skewed
        # strided-AP reduce then forms a[l] = t0[l-1] + t1[l] + t2[l+1].
        Sk8 = big.tile([BC, KW, L + 3], f32, tag="sk8")
        nc.vector.memset(Sk8[:, 0, 0:1], 0.0)          # t0[-1] = 0
        nc.vector.memset(Sk8[:, 2, L + 1:L + 2], 0.0)  # t2[L]  = 0
        kern_r = kern.rearrange("p (w cc) b -> p cc w b", w=KW)
        for b_ in range(BC):
            ksb = []
            for cc in range(C // 128):
                kt = kpool.tile([128, L], f32r, tag="k")
                nc.sync.dma_start(
                    out=kt, in_=kT[b_, cc * 128:(cc + 1) * 128, :].bitcast(f32r)
                )
                ksb.append(kt)
            scv = sa.tile([KW, L], f32, tag="scv")
            for lc in range(L // 512):
                cv = pcv.tile([KW, 512], f32, tag="cv")
                for cc in range(C // 128):
                    nc.tensor.matmul(
                        cv,
                        kern_r[:, cc, :, b_],
                        ksb[cc][:, lc * 512:(lc + 1) * 512],
                        start=(cc == 0), stop=(cc == C // 128 - 1),
                    )
                nc.scalar.copy(out=scv[:, lc * 512:(lc + 1) * 512], in_=cv)
            # partition-scatter: [3, L] rows -> single partition b_. Issued
            # from the scalar engine's HWDGE ring so the sync ring (k/v input
            # streams) never head-of-line blocks on compute-dependent stores.
            nc.scalar.dma_start(out=Sk8[b_:b_ + 1, :, 1:L + 1], in_=scv)

        # 3-tap combine: the taps live in each partition's free dim, so the
        # +/-1 shifts are plain free-dim offsets
        nc.vector.tensor_add(
            out=A_sb, in0=Sk8[:, 0, 0:L], in1=Sk8[:, 1, 1:L + 1]
        )
        nc.vector.tensor_add(out=A_sb, in0=A_sb, in1=Sk8[:, 2, 2:L + 2])

        # ---- masked softmax over l (b on partitions 0..7) ----
        # Am reuses Sk8's slot (released by the reduce); becomes e_ij in place
        Am = big.tile([BC, L], f32, tag="sk8")
        E_sb = Am
        nc.vector.tensor_add(out=Am, in0=A_sb, in1=maskf)
        nc.vector.tensor_reduce(
            out=nmx, in_=Am, op=mybir.AluOpType.max,
            axis=mybir.AxisListType.X, negate=True,
        )
        nc.scalar.activation(
            out=E_sb, in_=Am, func=mybir.ActivationFunctionType.Exp,
            bias=nmx[:, 0:1], scale=1.0, accum_out=ssum,
        )
        nc.vector.reciprocal(out=sinv, in_=ssum)
        nc.vector.tensor_scalar_mul(out=E_sb, in0=E_sb, scalar1=sinv[:, 0:1])

        nc.scalar.dma_start(out=a_out[:], in_=A_sb)
        nc.scalar.dma_start(out=e_out[:], in_=E_sb)

        # ---- ET[p, lt, b] = E[b, lt*128+p] (PE transpose, cast to bf16) ----
        for lt in range(L // 128):
            trp = ptr.tile([128, BC], f32, tag="tr")
            nc.tensor.transpose(trp, E_sb[:, lt * 128:(lt + 1) * 128], ident[0:BC, 0:BC])
            nc.vector.tensor_copy(out=ET[:, lt, :], in_=trp)

        # ---- attend[b, :] = sum_l e[l, b] * v[l, b, :] ----
        # lhsT = ET[:, lt, :] gives an [8, 512] output whose row m pairs
        # e(:, m) with v(:, b); only row b is real — DMA just that row out.
        for b_ in range(BC):
            vsb = []
            for jj in range(2):
                vt = vpool.tile([128, 8, V], bf16, tag="v")
                # on the sync ring AFTER the whole k stream: v intentionally
                # does not compete with k for bandwidth (conv->softmax is the
                # serial prefix; prefetching v early would delay it), and with
                # outputs moved to the scalar ring nothing blocks v during
                # the softmax window
                nc.sync.dma_start(
                    out=vt,
                    in_=vT[b_, jj * 1024:(jj + 1) * 1024, :].rearrange(
                        "(i p) v -> p i v", p=128
                    ),
                )
                vsb.append(vt)
            att_ps = pat.tile([BC, V], f32, tag="atp")
            for lt in range(L // 128):
                nc.tensor.matmul(
                    att_ps,
                    ET[:, lt, :],
                    vsb[lt // 8][:, lt % 8, :],
                    start=(lt == 0), stop=(lt == L // 128 - 1),
                )
            att_sb = sa.tile([BC, V], f32, tag="attsb")
            nc.vector.tensor_copy(out=att_sb, in_=att_ps)
            nc.scalar.dma_start(out=t_out[b_:b_ + 1, :], in_=att_sb[b_:b_ + 1, :])

    nc.compile()
    return nc


def get_nc():
    global _NC
    if _NC is None:
        _NC = _build()
    return _NC


def make_in_maps(q, k, v, k_mask, W, b):
    import ml_dtypes

    q = np.ascontiguousarray(np.asarray(q, dtype=np.float32))
    k = np.asarray(k, dtype=np.float32)
    v = np.asarray(v, dtype=np.float32)
    k_mask = np.asarray(k_mask, dtype=np.int32)
    W = np.asarray(W, dtype=np.float32)
    b = np.asarray(b, dtype=np.float32)

    # W column reorder: Wr[q, w*C + c] = W[c*KW + w, q]
    Wr = np.ascontiguousarray(W.reshape(C, KW, Q).transpose(2, 1, 0).reshape(Q, KW * C))
    Br = np.ascontiguousarray(b.reshape(C, KW).T.reshape(KW * C).reshape(M12, 128).T)

    in_maps = []
    for i in range(NCORES):
        sl = slice(i * BC, (i + 1) * BC)
        in_maps.append({
            "kT": np.ascontiguousarray(k[:, sl, :].transpose(1, 2, 0)),
            "vT": np.ascontiguousarray(v[:, sl, :].transpose(1, 0, 2)).astype(
                ml_dtypes.bfloat16
            ),
            "mT": np.ascontiguousarray(k_mask[:, sl].T),
            "qT": np.ascontiguousarray(q[sl, :].T),
            "Wr": Wr,
            "Br": Br,
        })
    return in_maps


def assemble(results):
    a = np.concatenate([r["a_out"] for r in results], axis=0).T.copy()
    e = np.concatenate([r["e_out"] for r in results], axis=0).T.copy()
    att = np.concatenate([r["t_out"] for r in results], axis=0)
    return (
        np.ascontiguousarray(a, dtype=np.float32),
        np.ascontiguousarray(e, dtype=np.float32),
        np.ascontiguousarray(att, dtype=np.float32),
    )


def kernel(q, k, v, k_mask, W, b):
    from concourse.bass_utils import run_bass_kernel_spmd

    nc = get_nc()
    in_maps = make_in_maps(q, k, v, k_mask, W, b)
    res = run_bass_kernel_spmd(nc, in_maps, core_ids=list(range(NCORES)))
    return assemble(res.results)
